# revision 18
# baseline (speedup 1.0000x reference)
"""Trainium2 Bass kernel for nn_BaseTransformer (ensemble member-attention block).

Sharding: data-parallel over batch B=8 across 8 NeuronCores (1 batch each).
Weights/constants replicated. No collectives.

Reference math (per batch b):
  value = einsum('ichw,oc->iohw', x, Wv)
  key   = selu(einsum(x, Wk)); query = selu(einsum(x, Wq))
  gram[c,i,j] = sum_s key[i,c,s] query[j,c,s] / 64
  A = softmax(gram, axis=i) + I
  transformed[j] = mean_i(value) + sum_i A[c,i,j] (value_i - mean)
                 = sum_i (A[c,i,j] - 1/16) value_i          (exact fold)
  out = selu(x + einsum(transformed, w_out) + b_out)

Key layout facts:
  K=16 members, C=HEADS=64, S=H*W=4096 spatial.
  pi(h, i) = 64*(i//8) + 8*h + (i%8)  -- partition permutation for the
  block-diagonal mix matmul operands (head-group h=0..7, member i=0..15).
"""

import sys

if "/opt/trn_rl_repo" not in sys.path:
    sys.path.insert(0, "/opt/trn_rl_repo")

import numpy as np

import concourse.bass as bass
import concourse.bacc as bacc
import concourse.mybir as mybir
import concourse.tile as tile

F32 = mybir.dt.float32

K, C, HEADS, S = 16, 64, 64, 4096
NG = 8          # head groups of 8
SC1 = 128       # phase-1 spatial chunk (gram contraction tile)
NCH1 = S // SC1  # 32
SC2 = 512       # phase-2 spatial chunk
NCH2 = S // SC2  # 8

ALPHA = 1.6732632423543772
LAMBDA = 1.0507009873554805
LN_ALPHA = float(np.log(ALPHA))
LN_LAMBDA_ALPHA = float(np.log(LAMBDA * ALPHA))
GRAM_SCALE = float(LAMBDA * LAMBDA / 64.0)


def _pi(h, i):
    return 64 * (i // 8) + 8 * h + (i % 8)


def host_constants(w_value, w_key, w_query, w_out, b_out):
    """Build all replicated device inputs on the host."""
    consts = {}
    # sigma: head c = 8u+g  <->  storage position 8g+u (group-contiguous).
    sigma = np.zeros(64, np.int64)
    for u in range(8):
        for g in range(8):
            sigma[8 * g + u] = 8 * u + g
    wvT = np.ascontiguousarray(w_value.T[:, sigma])          # cols in sigma order
    consts["wvT"] = np.concatenate([wvT, wvT], axis=0)       # [128, o] replicated
    wkqT = np.ascontiguousarray(np.concatenate([w_key.T, w_query.T], axis=1))
    consts["wkqT"] = np.concatenate([wkqT, wkqT], axis=0)    # [128, 128] replicated
    woutT = np.ascontiguousarray(w_out.T[sigma, :])          # rows in sigma order
    consts["woutT"] = np.concatenate([woutT, woutT], axis=0)  # [128, o] replicated
    i64 = np.eye(64, dtype=np.float32)
    consts["ident64"] = np.concatenate([i64, i64], axis=0)   # [128, 64] replicated
    consts["ident128"] = np.eye(128, dtype=np.float32)

    # Gram psum layout: partition = j*8 + u (q side), free = i*8 + u' (k side),
    # where u indexes the head within stride-8 group g (head c = 8u + g).
    # MASK zeroes cross-head entries (u != u').
    mask = np.zeros((128, 128), np.float32)
    for p in range(128):
        for f in range(128):
            if p % 8 == f % 8:
                mask[p, f] = 1.0
    consts["maskg"] = mask

    # P (mm2 lhsT): rows r=(i,u)=8i+u -> out partition pi(u, i)
    P = np.zeros((128, 128), np.float32)
    for u in range(8):
        for i in range(16):
            P[8 * i + u, _pi(u, i)] = 1.0
    consts["permP"] = P
    # P' (mm1 rhs): rows p=(j,u)=8j+u -> col pi(u, j)
    consts["permPp"] = P.copy()

    # DPAT in permuted coords: D[pi(h,i), pi(h,j)] = delta(i,j) - 1/16,
    # D[cross-head] = -0 (only same-head blocks get the -1/16 + identity).
    D = np.zeros((128, 128), np.float32)
    for h in range(8):
        for i in range(16):
            for j in range(16):
                D[_pi(h, i), _pi(h, j)] = (1.0 if i == j else 0.0) - 1.0 / 16.0
    consts["dpat"] = D

    be = np.concatenate([b_out, b_out]).astype(np.float32) + LN_LAMBDA_ALPHA
    consts["bias_exp"] = be.reshape(128, 1)
    br = (LAMBDA * np.concatenate([b_out, b_out])).astype(np.float32)
    consts["bias_relu"] = br.reshape(128, 1)
    return consts


def build_nc():
    """Build the single-core Bass program (same NEFF on all 8 cores)."""
    nc = bacc.Bacc("TRN2", target_bir_lowering=False, debug=False)

    x_d = nc.dram_tensor("x", [K, C, S], F32, kind="ExternalInput")
    wvT_d = nc.dram_tensor("wvT", [128, 64], F32, kind="ExternalInput")
    wkqT_d = nc.dram_tensor("wkqT", [128, 128], F32, kind="ExternalInput")
    woutT_d = nc.dram_tensor("woutT", [128, 64], F32, kind="ExternalInput")
    i64_d = nc.dram_tensor("ident64", [128, 64], F32, kind="ExternalInput")
    i128_d = nc.dram_tensor("ident128", [128, 128], F32, kind="ExternalInput")
    mask_d = nc.dram_tensor("maskg", [128, 128], F32, kind="ExternalInput")
    permP_d = nc.dram_tensor("permP", [128, 128], F32, kind="ExternalInput")
    permPp_d = nc.dram_tensor("permPp", [128, 128], F32, kind="ExternalInput")
    dpat_d = nc.dram_tensor("dpat", [128, 128], F32, kind="ExternalInput")
    be_d = nc.dram_tensor("bias_exp", [128, 1], F32, kind="ExternalInput")
    br_d = nc.dram_tensor("bias_relu", [128, 1], F32, kind="ExternalInput")
    out_d = nc.dram_tensor("out", [K, C, S], F32, kind="ExternalOutput")

    with tile.TileContext(nc) as tc:
        with (
            tc.tile_pool(name="persist", bufs=1) as persist,
            tc.tile_pool(name="xpool", bufs=1) as xpool,
        ):
            # ---- weights / constants to SBUF ----
            wv_sb = persist.tile([128, 64], F32, tag="wv")
            nc.sync.dma_start(out=wv_sb, in_=wvT_d[:, :])
            wkq_sb = persist.tile([128, 128], F32, tag="wkq")
            nc.sync.dma_start(out=wkq_sb, in_=wkqT_d[:, :])
            wo_sb = persist.tile([128, 64], F32, tag="wo")
            nc.sync.dma_start(out=wo_sb, in_=woutT_d[:, :])
            i64_sb = persist.tile([128, 64], F32, tag="i64")
            nc.sync.dma_start(out=i64_sb, in_=i64_d[:, :])
            i128_sb = persist.tile([128, 128], F32, tag="i128")
            nc.sync.dma_start(out=i128_sb, in_=i128_d[:, :])
            mask_sb = persist.tile([128, 128], F32, tag="mask")
            nc.sync.dma_start(out=mask_sb, in_=mask_d[:, :])
            permP_sb = persist.tile([128, 128], F32, tag="permP")
            nc.sync.dma_start(out=permP_sb, in_=permP_d[:, :])
            permPp_sb = persist.tile([128, 128], F32, tag="permPp")
            nc.sync.dma_start(out=permPp_sb, in_=permPp_d[:, :])
            dpat_sb = persist.tile([128, 128], F32, tag="dpat")
            nc.sync.dma_start(out=dpat_sb, in_=dpat_d[:, :])
            be_sb = persist.tile([128, 1], F32, tag="be")
            nc.sync.dma_start(out=be_sb, in_=be_d[:, :])
            br_sb = persist.tile([128, 1], F32, tag="br")
            nc.sync.dma_start(out=br_sb, in_=br_d[:, :])
            lna_sb = persist.tile([128, 1], F32, tag="lna")
            nc.vector.memset(lna_sb, LN_ALPHA)
            zero_sb = persist.tile([128, 1], F32, tag="zero")
            nc.vector.memset(zero_sb, 0.0)

            # ---- x resident: 8 pair tiles [128, S], pair t = members (t, t+8) ----
            x_sb = []
            for t in range(8):
                xt = xpool.tile([128, S], F32, tag=f"x{t}")
                nc.sync.dma_start(out=xt[0:64, :], in_=x_d[t, :, :])
                nc.sync.dma_start(out=xt[64:128, :], in_=x_d[t + 8, :, :])
                x_sb.append(xt)

            # BigB result tiles (persist into phase 2)
            bigB = []
            for g in range(NG):
                bigB_t = persist.tile([128, 128], F32, tag=f"bigB{g}")
                bigB.append(bigB_t)

            # =========================== PHASE 1 ===========================
            with (
                tc.tile_pool(name="p1sb", bufs=2) as p1sb,
                tc.tile_pool(name="p1sc", bufs=2) as p1sc,
                tc.tile_pool(name="kqps", bufs=2, space="PSUM") as kqps,
                tc.tile_pool(name="gramps", bufs=1, space="PSUM") as gramps,
            ):
                # gram psum: 2 tensors x [128, 512], 4 groups each
                gram_ps = []
                for gb in range(2):
                    gram_t = gramps.tile([128, 512], F32, tag=f"gram{gb}")
                    gram_ps.append(gram_t)

                for sc in range(NCH1):
                    sl = slice(SC1 * sc, SC1 * (sc + 1))
                    kqT = p1sb.tile([128, K * 128], F32, tag="kqT")
                    for blk in range(2):  # member blocks [0..8), [8..16)
                        ps = kqps.tile([128, 8 * 128], F32, tag="kqps")
                        for mb in range(8):
                            m = blk * 8 + mb
                            xt = x_sb[m % 8]
                            rhalf = slice(0, 64) if m < 8 else slice(64, 128)
                            nc.tensor.matmul(
                                ps[:, 128 * mb: 128 * (mb + 1)],
                                xt[rhalf, sl], wkq_sb[rhalf, :],
                                start=True, stop=True,
                            )
                        # selu (without lambda; lambda^2 folded into gram exp):
                        # e2 = exp(kq + ln a); r = relu(kq); out = (e2 - a) min r
                        e2 = p1sc.tile([128, 8 * 128], F32, tag="e2")
                        nc.scalar.activation(
                            out=e2, in_=ps,
                            func=mybir.ActivationFunctionType.Exp,
                            bias=lna_sb[:, 0:1])
                        r = p1sc.tile([128, 8 * 128], F32, tag="r")
                        nc.scalar.activation(
                            out=r, in_=ps,
                            func=mybir.ActivationFunctionType.Relu,
                            bias=zero_sb[:, 0:1])
                        # kqT free layout: half*1024 + m*64 + c
                        ev = e2.rearrange("p (mb half c) -> p half mb c",
                                          mb=8, half=2, c=64)
                        rv = r.rearrange("p (mb half c) -> p half mb c",
                                         mb=8, half=2, c=64)
                        for half in range(2):
                            nc.vector.scalar_tensor_tensor(
                                out=kqT[:, 1024 * half + 512 * blk:
                                        1024 * half + 512 * (blk + 1)],
                                in0=ev[:, half], scalar=ALPHA, in1=rv[:, half],
                                op0=mybir.AluOpType.subtract,
                                op1=mybir.AluOpType.min)
                    # gram matmuls: lhsT = q side (M = 8j+u), rhs = k side
                    # (N = 8i+u'): single-stride [[8,128]] APs, offset g
                    vq = kqT.rearrange("p (f e) -> p e f", f=256, e=8)
                    for g in range(NG):
                        q_ap = vq[:, g, 128:256]
                        k_ap = vq[:, g, 0:128]
                        # 4 grams share one psum bank = one zero region: the
                        # accumulation group is started by the first gram of the
                        # bank at sc==0 and stopped by the last at the final sc.
                        nc.tensor.matmul(
                            gram_ps[g // 4][:, 128 * (g % 4): 128 * (g % 4 + 1)],
                            q_ap, k_ap,
                            start=(sc == 0 and g % 4 == 0),
                            stop=(sc == NCH1 - 1 and g % 4 == 3))

                # ---- softmax (no max-sub; range pre-verified) + BigB build ----
                for g in range(NG):
                    gp = gram_ps[g // 4][:, 128 * (g % 4): 128 * (g % 4 + 1)]
                    E = p1sc.tile([128, 128], F32, tag="E")
                    nc.scalar.activation(
                        out=E, in_=gp,
                        func=mybir.ActivationFunctionType.Exp,
                        bias=zero_sb[:, 0:1], scale=GRAM_SCALE)
                    Ssum = p1sc.tile([128, 8], F32, tag="Ssum")
                    nc.vector.tensor_reduce(
                        out=Ssum,
                        in_=E.rearrange("p (i u) -> p u i", i=16, u=8),
                        axis=mybir.AxisListType.X, op=mybir.AluOpType.add)
                    R = p1sc.tile([128, 8], F32, tag="R")
                    nc.vector.reciprocal(out=R, in_=Ssum)
                    Eu = E.rearrange("p (i u) -> p u i", i=16, u=8)
                    for u in range(8):
                        nc.vector.tensor_scalar(
                            out=Eu[:, u, :], in0=Eu[:, u, :],
                            scalar1=R[:, u: u + 1], scalar2=None,
                            op0=mybir.AluOpType.mult)
                    # mask cross-head, then BigB = P^T (W masked)^T P' + DPAT
                    nc.vector.tensor_tensor(
                        out=E, in0=E, in1=mask_sb, op=mybir.AluOpType.mult)
                    c_ps = kqps.tile([128, 128], F32, tag="kqps")
                    nc.tensor.matmul(c_ps, E, permPp_sb, start=True, stop=True)
                    c_sb = p1sc.tile([128, 128], F32, tag="permcsb")
                    nc.scalar.copy(c_sb, c_ps)
                    b_ps = kqps.tile([128, 128], F32, tag="kqps")
                    nc.tensor.matmul(b_ps, permP_sb, c_sb, start=True, stop=True)
                    nc.vector.scalar_tensor_tensor(
                        out=bigB[g], in0=b_ps, scalar=1.0, in1=dpat_sb,
                        op0=mybir.AluOpType.mult, op1=mybir.AluOpType.add)

            # =========================== PHASE 2 ===========================
            with (
                tc.tile_pool(name="vflat", bufs=1) as vflatp,
                tc.tile_pool(name="tflat", bufs=1) as tflatp,
                tc.tile_pool(name="p2sc", bufs=2) as p2sc,
                tc.tile_pool(name="p2out", bufs=2) as p2outp,
                tc.tile_pool(name="vps", bufs=2, space="PSUM") as vps,
                tc.tile_pool(name="mps", bufs=2, space="PSUM") as mps,
                tc.tile_pool(name="ops", bufs=2, space="PSUM") as ops,
            ):
                prev_gathers = []
                prev_last_conv2 = None
                for pc in range(NCH2):
                    sl = slice(SC2 * pc, SC2 * (pc + 1))
                    # --- value conv into pair psum, copy to vflat2 ---
                    # vflat2 [128, 8*SC2]: row = 64*(i//8) + c, free = (i%8)*SC2 + s
                    vflat2 = vflatp.tile([128, 8 * SC2], F32, tag="vflat2")
                    vcopies = []
                    for t in range(8):
                        ps = vps.tile([128, SC2], F32, tag="vps")
                        nc.tensor.matmul(ps[0:64, :], wv_sb[0:64, :],
                                         x_sb[t][0:64, sl], start=True, stop=True)
                        nc.tensor.matmul(ps[64:128, :], wv_sb[64:128, :],
                                         x_sb[t][64:128, sl], start=True, stop=True)
                        vci = nc.scalar.copy(vflat2[:, SC2 * t: SC2 * (t + 1)], ps)
                        vcopies.append(vci)
                        # WAR: vflat2 slot is reused (bufs=1); the strided gather
                        # reads of the previous chunk are not range-tracked, so
                        # order them explicitly before this chunk's first write.
                        if t == 0:
                            for gi_prev in prev_gathers:
                                tile.add_dep_helper(
                                    vci.ins, gi_prev.ins,
                                    reason="vflat2 WAR vs prev gathers")
                    prev_gathers = []
                    # --- mix: gather -> blockdiag matmul -> copy ---
                    # tflat2 [128, 8*SC2]: row = 64*(j//8) + c, free = (j%8)*SC2 + s
                    tflat2 = tflatp.tile([128, 8 * SC2], F32, tag="tflat2")
                    vv = vflat2.rearrange("p (it s) -> p it s", it=8, s=SC2)
                    scatters = []
                    for g in range(NG):
                        vg = p2sc.tile([128, SC2], F32, tag="vg")
                        # rows [64*i2+8u+it] <- src part (64*i2 + 8u + g), free it*SC2+s
                        for i2 in range(2):
                            base = 64 * i2 + 8 * g
                            gi = nc.sync.dma_start(
                                out=vg[64 * i2: 64 * (i2 + 1), :],
                                in_=vv[base: base + 8, :, :])
                            # gather reads vflat2 through a strided view that the
                            # tile tracker misses: order after the value copies.
                            tile.add_dep_helper(
                                gi.ins, vcopies[-1].ins,
                                reason="gather after value copies")
                            prev_gathers.append(gi)
                        pm = mps.tile([128, SC2], F32, tag="mps")
                        nc.tensor.matmul(pm, bigB[g], vg, start=True, stop=True)
                        mg = p2sc.tile([128, SC2], F32, tag="mg")
                        nc.scalar.copy(mg, pm)
                        # scatter: src rows [64*j2 + 8u + jt] ->
                        # dst rows 64*j2 + 8u + g, free jt*SC2 + s
                        tv = tflat2.rearrange("p (jt s) -> p jt s", jt=8, s=SC2)
                        for j2 in range(2):
                            base = 64 * j2 + 8 * g
                            si = nc.sync.dma_start(
                                out=tv[base: base + 8, :, :],
                                in_=mg[64 * j2: 64 * (j2 + 1), :])
                            scatters.append(si)
                            # WAR: tflat2 slot reuse vs prev chunk's conv2 reads
                            if prev_last_conv2 is not None:
                                tile.add_dep_helper(
                                    si.ins, prev_last_conv2.ins,
                                    reason="tflat2 WAR vs prev conv2")
                    # --- conv2 + x-add + final selu + out DMA, pairs (jt, jt+8) ---
                    for jt in range(8):
                        po = ops.tile([128, SC2], F32, tag="ops")
                        mi = nc.tensor.matmul(
                            po[0:64, :], wo_sb[0:64, :],
                            tflat2[0:64, SC2 * jt: SC2 * (jt + 1)],
                            start=True, stop=False)
                        if jt == 0:
                            # conv2 reads tflat2, whose scatter writes are not
                            # range-tracked: order after all scatters (PE FIFO
                            # extends this to the later conv2 matmuls).
                            for si in scatters:
                                tile.add_dep_helper(
                                    mi.ins, si.ins,
                                    reason="conv2 after scatters")
                        nc.tensor.matmul(po[0:64, :], i64_sb[0:64, :],
                                         x_sb[jt][0:64, sl],
                                         start=False, stop=True)
                        nc.tensor.matmul(po[64:128, :], wo_sb[64:128, :],
                                         tflat2[64:128, SC2 * jt: SC2 * (jt + 1)],
                                         start=True, stop=False)
                        prev_last_conv2 = nc.tensor.matmul(
                            po[64:128, :], i64_sb[64:128, :],
                            x_sb[jt][64:128, sl],
                            start=False, stop=True)
                        e2 = p2sc.tile([128, SC2], F32, tag="fe2")
                        nc.scalar.activation(
                            out=e2, in_=po,
                            func=mybir.ActivationFunctionType.Exp,
                            bias=be_sb[:, 0:1])
                        r2 = p2sc.tile([128, SC2], F32, tag="fr2")
                        nc.scalar.activation(
                            out=r2, in_=po,
                            func=mybir.ActivationFunctionType.Relu,
                            bias=br_sb[:, 0:1], scale=LAMBDA)
                        o_sb = p2outp.tile([128, SC2], F32, tag="osb")
                        nc.vector.scalar_tensor_tensor(
                            out=o_sb, in0=e2, scalar=float(LAMBDA * ALPHA), in1=r2,
                            op0=mybir.AluOpType.subtract, op1=mybir.AluOpType.min)
                        nc.sync.dma_start(out=out_d[jt, :, sl], in_=o_sb[0:64, :])
                        nc.sync.dma_start(out=out_d[jt + 8, :, sl], in_=o_sb[64:128, :])
    nc.compile()
    return nc


_NC_CACHE = None


def _get_nc():
    global _NC_CACHE
    if _NC_CACHE is None:
        _NC_CACHE = build_nc()
    return _NC_CACHE


def kernel(in_tensor, w_value, w_key, w_query, w_out, b_out, **_ignored):
    in_tensor = np.asarray(in_tensor, dtype=np.float32)
    w_value = np.asarray(w_value, dtype=np.float32)
    w_key = np.asarray(w_key, dtype=np.float32)
    w_query = np.asarray(w_query, dtype=np.float32)
    w_out = np.asarray(w_out, dtype=np.float32)
    b_out = np.asarray(b_out, dtype=np.float32)

    B = in_tensor.shape[0]
    assert B == 8
    consts = host_constants(w_value, w_key, w_query, w_out, b_out)

    nc = _get_nc()
    in_maps = []
    for b in range(B):
        m = {"x": np.ascontiguousarray(in_tensor[b].reshape(K, C, S))}
        m.update(consts)
        in_maps.append(m)

    from concourse.bass_utils import run_bass_kernel_spmd

    res = run_bass_kernel_spmd(nc, in_maps, core_ids=list(range(8)))
    outs = [res.results[b]["out"].reshape(K, C, 64, 64) for b in range(B)]
    return np.stack(outs, axis=0).astype(np.float32)


if __name__ == "__main__":
    # quick smoke: build only
    nc = build_nc()
    print("built ok; instructions:",
          sum(1 for _ in nc.m.functions[0].instructions)
          if hasattr(nc.m.functions[0], "instructions") else "?")
